# revision 39
# baseline (speedup 1.0000x reference)
"""Multi-head attention (b=2, n=2048, dim=1024, h=16, dh=64) on 8 TRN2 NeuronCores.

Sharding: 32 (batch, head) pairs -> 8 cores x (1 batch, 4 heads). No collectives.
Per core:
  inputs : xT  [1024, 2048] bf16  (x[b].T, k-major so K sits on SBUF partitions)
           wq  [1024, 256]  bf16  (q-columns of w_qkv for this core's 4 heads, pre-scaled by 1/8)
           wk  [1024, 256]  bf16
           wv  [1024, 256]  bf16
  output : out [4*65, 2048] f32   (per local head: rows 0-63 = unnormalized (attn@v)^T,
                                   row 64 = softmax denominator per query)
Host divides by the denominator and transposes back to [b, n, h*dh].

Device pipeline per core:
  qT/kT = (w.T @ x.T) in [d, n] layout, head-pairs packed 2x64 on partitions (bf16)
  V     = (x @ wv)    in [n, d] layout with a ones column appended (bf16)
  per head pair, per 512-wide query chunk, per 128-wide key block:
    S^T[j,i] = kT.T @ qT   (two K=64 matmuls packed into PE row-groups 0-63 / 64-127)
    A^T      = exp(S^T)    (one ACT instr over both heads' PSUM banks, f32 -> bf16)
    O^T     += [V|1].T @ A^T  (PSUM-accumulated over key blocks; row 64 = rowsum)
"""

import numpy as np
import ml_dtypes

B, N, DIM = 2, 2048, 1024
HEADS, DH = 16, 64
P = 128
KT = DIM // P          # 8 k-tiles
NT = N // P            # 16 n/j blocks
NCH = N // 512         # 4 chunks of 512
HL = 4                 # local heads per core
OROWS = HL * (DH + 1)  # 260 output rows per core

_CACHE = {}
LAST_RESULTS = None
TRACE = False


def _build_nc():
    from contextlib import ExitStack

    import concourse.bass as bass
    import concourse.tile as tile
    from concourse import bacc, mybir

    bf16 = mybir.dt.bfloat16
    fp16 = mybir.dt.float16
    f32 = mybir.dt.float32

    nc = bacc.Bacc("TRN2", target_bir_lowering=False)

    xT_d = nc.dram_tensor("xT", [DIM, N], bf16, kind="ExternalInput")
    wq_d = nc.dram_tensor("wq", [DIM, HL * DH], bf16, kind="ExternalInput")
    wk_d = nc.dram_tensor("wk", [DIM, HL * DH], bf16, kind="ExternalInput")
    wv_d = nc.dram_tensor("wv", [DIM, HL * DH], bf16, kind="ExternalInput")
    out_d = nc.dram_tensor("out", [OROWS, N], f32, kind="ExternalOutput")

    # out rows viewed as [row-within-head, head, n] for packed output DMAs
    out_r = out_d[:, :].rearrange("(hh r) n -> r hh n", r=DH + 1)
    xT_r = xT_d[:, :].rearrange("(kt p) n -> p kt n", p=P)
    wq_r = wq_d[:, :].rearrange("(kt p) c -> p kt c", p=P)
    wk_r = wk_d[:, :].rearrange("(kt p) c -> p kt c", p=P)
    wv_r = wv_d[:, :].rearrange("(kt p) c -> p kt c", p=P)

    with tile.TileContext(nc) as tc, ExitStack() as ctx:
        sing = ctx.enter_context(tc.tile_pool(name="sing", bufs=1))
        spool = ctx.enter_context(
            tc.tile_pool(name="s_ps", bufs=3, space=bass.MemorySpace.PSUM)
        )
        opool = ctx.enter_context(
            tc.tile_pool(name="o_ps", bufs=1, space=bass.MemorySpace.PSUM)
        )
        apool = ctx.enter_context(tc.tile_pool(name="a_sb", bufs=14))
        copool = ctx.enter_context(tc.tile_pool(name="o_sb", bufs=4))

        # persistent SBUF tensors
        xT = sing.tile([P, KT, N], bf16, tag="xT")
        wq = sing.tile([P, KT, HL * DH], bf16, tag="wq")
        wk = sing.tile([P, KT, HL * DH], bf16, tag="wk")
        wv = sing.tile([P, KT, HL * DH], bf16, tag="wv")
        # head-pair packed projections: partitions 0-63 head A dims, 64-127 head B
        qT = [sing.tile([P, N], bf16, tag=f"qT{i}", name=f"qT{i}") for i in range(2)]
        kT = [sing.tile([P, N], bf16, tag=f"kT{i}", name=f"kT{i}") for i in range(2)]
        # V in [j, d] layout per j-block per head, with ones column at d=64
        v = sing.tile([P, NT, HL, DH + 1], bf16, tag="v")

        # input DMAs
        nc.gpsimd.dma_start(out=wk[:], in_=wk_r[:])
        nc.gpsimd.dma_start(out=wq[:], in_=wq_r[:])
        nc.gpsimd.dma_start(out=wv[:], in_=wv_r[:])
        for c in range(4):
            nc.sync.dma_start(
                out=xT[:, 2 * c : 2 * c + 2, :], in_=xT_r[:, 2 * c : 2 * c + 2, :]
            )



        # ---- projections ----
        # k, q: out[c, n] = w[:, c].T @ xT.  hp0 upfront; hp1 woven into
        # attention-hp0's periods (PE fills slack while ACT runs exp).
        def proj_unit(wt, dst, hp, nch):
            """Emit the 8 K-accumulated matmuls + copy for one 512-col chunk,
            returned as two 4-matmul halves so weaving stays fine-grained."""
            state = {}

            def half(h):
                if h == 0:
                    state["ps"] = spool.tile([P, 512], f32, tag="sp", name="ps")
                ps = state["ps"]
                for kt in range(4 * h, 4 * h + 4):
                    nc.tensor.matmul(
                        ps[:],
                        wt[:, kt, hp * P : (hp + 1) * P],
                        xT[:, kt, nch * 512 : (nch + 1) * 512],
                        start=(kt == 0),
                        stop=(kt == KT - 1),
                    )
                if h == 1:
                    nc.vector.tensor_copy(dst[:, nch * 512 : (nch + 1) * 512], ps[:])

            return [lambda: half(0), lambda: half(1)]

        # PE warm-up: zero matmuls while the xT DMA streams in, so the HAM
        # clock-gate is at 8/8 when real work starts (idle PE decays to 1.2GHz)
        zbuf = sing.tile([P, 512], bf16, tag="zbuf")
        nc.vector.memset(zbuf[:], 0.0)
        # ones column of V (softmax denominator comes out of the PV matmul)
        nc.vector.memset(v[:, :, :, DH : DH + 1], 1.0)
        zp = spool.tile([P, 512], f32, tag="sp", name="zp")
        for _ in range(28):
            nc.tensor.matmul(zp[:], zbuf[:, 0:P], zbuf[:], start=True, stop=True)

        for unit in [proj_unit(wk, kT[0], 0, 0), proj_unit(wq, qT[0], 0, 0)]:
            for work in unit:
                work()

        # remaining projections are woven into the attention periods; each
        # woven chunk lands (in emission order) before the first scores
        # matmul that reads it.
        def full_unit(halves):
            return lambda: [h() for h in halves]

        woven = [full_unit(proj_unit(wq, qT[0], 0, 1))]
        woven_rest = []
        for nch in range(2, NCH):
            woven_rest.append(full_unit(proj_unit(wq, qT[0], 0, nch)))
        for wt, dst in ((wk, kT[1]), (wq, qT[1])):
            for nch in range(NCH):
                woven_rest.append(full_unit(proj_unit(wt, dst, 1, nch)))

        # V: out[n, c] = xT[:, ntile].T @ wv   -> [128 n, 256 c]
        def v_unit(nt):
            state = {}

            def half(h):
                if h == 0:
                    state["ps"] = spool.tile([P, HL * DH], f32, tag="sp", name="psv")
                ps = state["ps"]
                for kt in range(4 * h, 4 * h + 4):
                    nc.tensor.matmul(
                        ps[:],
                        xT[:, kt, nt * P : (nt + 1) * P],
                        wv[:, kt, :],
                        start=(kt == 0),
                        stop=(kt == KT - 1),
                    )
                if h == 1:
                    # scatter the 4 heads' 64 cols into the [NT, HL, 65] layout
                    nc.vector.tensor_copy(
                        v[:, nt, :, 0:DH],
                        ps[:].rearrange("p (h d) -> p h d", h=HL),
                    )

            return [lambda: half(0), lambda: half(1)]

        v_units = [full_unit(v_unit(nt)) for nt in range(NT)]

        # ---- attention ----
        # 8 blocks of 16 periods (one per (hp, ic)).  ACT runs one
        # [128, 1024] exp per period back-to-back; PE emits scores two
        # periods ahead (spool rotation) plus woven projection work; PV runs
        # as dense 8-matmul bursts every 4 periods (no exp-latency exposure).
        # Block 0 weaves the V projection (PV bursts shifted late until V is
        # ready); blocks 1+ weave the remaining q/k projections.
        blocks = [(hp, ic) for hp in range(2) for ic in range(NCH)]
        ats = {}
        opairs = {}
        sp_ahead = {}

        def emit_scores(b, jb):
            hp, ic = blocks[b]
            i0, j0 = ic * 512, jb * P
            sp = spool.tile([P, 1024], f32, tag="sp", name="sp")
            nc.tensor.matmul(
                sp[:, 0:512],
                kT[hp][0:DH, j0 : j0 + P],
                qT[hp][0:DH, i0 : i0 + 512],
                start=True, stop=True, tile_position=(0, 0),
            )
            nc.tensor.matmul(
                sp[:, 512:1024],
                kT[hp][DH:P, j0 : j0 + P],
                qT[hp][DH:P, i0 : i0 + 512],
                start=True, stop=True, tile_position=(64, 0),
            )
            return sp

        def emit_exp(b, jb, sp):
            at = apool.tile([P, 1024], bf16, tag="at", name="at")
            nc.scalar.activation(at[:], sp[:], mybir.ActivationFunctionType.Exp)
            ats[(b, jb)] = at

        def fetch_scores(b, jb):
            key = (b, jb)
            if key in sp_ahead:
                return sp_ahead.pop(key)
            return emit_scores(b, jb)

        def emit_pv_quarter(b, q):
            """PV matmuls for periods 4q..4q+3 of block b (dense burst)."""
            hp, ic = blocks[b]
            if q == 0:
                opairs[b] = (
                    opool.tile([DH + 1, 512], f32, tag="oA", name="oA"),
                    opool.tile([DH + 1, 512], f32, tag="oB", name="oB"),
                )
            oA, oB = opairs[b]
            for col, o in ((0, oA), (1, oB)):
                for jb in range(4 * q, 4 * q + 4):
                    nc.tensor.matmul(
                        o[:],
                        v[:, jb, 2 * hp + col, :],
                        ats[(b, jb)][:, 512 * col : 512 * col + 512],
                        start=(jb == 0), stop=(jb == NT - 1),
                    )
            for jb in range(4 * q, 4 * q + 4):
                del ats[(b, jb)]
            if q == 3:
                i0 = ic * 512
                os = copool.tile([DH + 1, 2, 512], f32, tag="os", name="os")
                nc.vector.tensor_copy(os[:, 0, :], oA[:])
                nc.vector.tensor_copy(os[:, 1, :], oB[:])
                nc.sync.dma_start(
                    out=out_r[:, 2 * hp : 2 * hp + 2, i0 : i0 + 512],
                    in_=os[:],
                )

        LA = 2  # scores lookahead depth
        nblocks = len(blocks)
        # prime the pipeline, then finish the kT01 projection chunks so the
        # first exp only waits on k01n0 + q01n0
        for j in range(LA):
            sp_ahead[(0, j)] = emit_scores(0, j)
        for nch in range(1, NCH):
            for work in proj_unit(wk, kT[0], 0, nch):
                work()
        for b in range(nblocks):
            for jb in range(NT):
                emit_exp(b, jb, fetch_scores(b, jb))
                la = jb + LA
                if la < NT:
                    sp_ahead[(b, la)] = emit_scores(b, la)
                elif b + 1 < nblocks:
                    sp_ahead[(b + 1, la - NT)] = emit_scores(b + 1, la - NT)
                # woven PE filler
                p = b * NT + jb
                if b == 0:
                    if woven:
                        woven.pop(0)()
                    for _ in range(2):
                        if not woven and v_units:
                            v_units.pop(0)()
                elif woven_rest and (p - NT) % 5 == 4:
                    woven_rest.pop(0)()
                # PV bursts (block 0 deferred until woven V is ready; each
                # block's last quarter runs in the next block's first period
                # so the boundary scores lookahead isn't delayed)
                if b == 0:
                    if jb in (7, 11):
                        emit_pv_quarter(0, (jb - 7) // 4)
                    elif jb == NT - 1:
                        while v_units:
                            v_units.pop(0)()
                        emit_pv_quarter(0, 2)
                        emit_pv_quarter(0, 3)
                elif jb % 4 == 3:
                    emit_pv_quarter(b, jb // 4)

    nc.compile()
    return nc


def _get_nc():
    if "nc" not in _CACHE:
        _CACHE["nc"] = _build_nc()
    return _CACHE["nc"]


def _prepare_in_maps(x, w_qkv):
    bf = ml_dtypes.bfloat16
    x = np.asarray(x, dtype=np.float32)
    w = np.asarray(w_qkv, dtype=np.float32)
    scale = DH ** -0.5
    in_maps = []
    xT_b = [np.ascontiguousarray(x[b].T).astype(bf) for b in range(B)]
    for c in range(8):
        b, hg = divmod(c, 4)
        cs = slice(hg * HL * DH, (hg + 1) * HL * DH)
        in_maps.append(
            {
                "xT": xT_b[b],
                "wq": np.ascontiguousarray(w[:, cs] * scale).astype(bf),
                "wk": np.ascontiguousarray(w[:, 1024:2048][:, cs]).astype(bf),
                "wv": np.ascontiguousarray(w[:, 2048:3072][:, cs]).astype(bf),
            }
        )
    return in_maps


def _assemble(outs):
    full = np.empty((B, N, HEADS * DH), dtype=np.float32)
    for c in range(8):
        b, hg = divmod(c, 4)
        o = outs[c].reshape(HL, DH + 1, N)
        norm = o[:, :DH, :] / o[:, DH : DH + 1, :]  # [hl, d, n]
        full[b, :, hg * HL * DH : (hg + 1) * HL * DH] = norm.transpose(2, 0, 1).reshape(
            N, HL * DH
        )
    return full


def kernel(x, w_qkv):
    global LAST_RESULTS
    from concourse.bass_utils import run_bass_kernel_spmd

    nc = _get_nc()
    in_maps = _prepare_in_maps(x, w_qkv)
    res = run_bass_kernel_spmd(
        nc,
        in_maps,
        core_ids=list(range(8)),
        trace=TRACE,
        trace_cores=[0] if TRACE else None,
    )
    LAST_RESULTS = res
    return _assemble([r["out"] for r in res.results])


# revision 40
# speedup vs baseline: 1.0021x; 1.0021x over previous
"""Multi-head attention (b=2, n=2048, dim=1024, h=16, dh=64) on 8 TRN2 NeuronCores.

Sharding: 32 (batch, head) pairs -> 8 cores x (1 batch, 4 heads). No collectives.
Per core:
  inputs : xT  [1024, 2048] bf16  (x[b].T, k-major so K sits on SBUF partitions)
           wq  [1024, 256]  bf16  (q-columns of w_qkv for this core's 4 heads, pre-scaled by 1/8)
           wk  [1024, 256]  bf16
           wv  [1024, 256]  bf16
  output : out [4*65, 2048] f32   (per local head: rows 0-63 = unnormalized (attn@v)^T,
                                   row 64 = softmax denominator per query)
Host divides by the denominator and transposes back to [b, n, h*dh].

Device pipeline per core:
  qT/kT = (w.T @ x.T) in [d, n] layout, head-pairs packed 2x64 on partitions (bf16)
  V     = (x @ wv)    in [n, d] layout with a ones column appended (bf16)
  per head pair, per 512-wide query chunk, per 128-wide key block:
    S^T[j,i] = kT.T @ qT   (two K=64 matmuls packed into PE row-groups 0-63 / 64-127)
    A^T      = exp(S^T)    (one ACT instr over both heads' PSUM banks, f32 -> bf16)
    O^T     += [V|1].T @ A^T  (PSUM-accumulated over key blocks; row 64 = rowsum)
"""

import numpy as np
import ml_dtypes

B, N, DIM = 2, 2048, 1024
HEADS, DH = 16, 64
P = 128
KT = DIM // P          # 8 k-tiles
NT = N // P            # 16 n/j blocks
NCH = N // 512         # 4 chunks of 512
HL = 4                 # local heads per core
OROWS = HL * (DH + 1)  # 260 output rows per core

_CACHE = {}
LAST_RESULTS = None
TRACE = False


def _build_nc():
    from contextlib import ExitStack

    import concourse.bass as bass
    import concourse.tile as tile
    from concourse import bacc, mybir

    bf16 = mybir.dt.bfloat16
    fp16 = mybir.dt.float16
    f32 = mybir.dt.float32

    nc = bacc.Bacc("TRN2", target_bir_lowering=False)

    xT_d = nc.dram_tensor("xT", [P, KT * N], bf16, kind="ExternalInput")
    wq_d = nc.dram_tensor("wq", [DIM, HL * DH], bf16, kind="ExternalInput")
    wk_d = nc.dram_tensor("wk", [DIM, HL * DH], bf16, kind="ExternalInput")
    wv_d = nc.dram_tensor("wv", [DIM, HL * DH], bf16, kind="ExternalInput")
    out_d = nc.dram_tensor("out", [OROWS, N], f32, kind="ExternalOutput")

    # out rows viewed as [row-within-head, head, n] for packed output DMAs
    out_r = out_d[:, :].rearrange("(hh r) n -> r hh n", r=DH + 1)
    xT_r = xT_d[:, :].rearrange("p (kt n) -> p kt n", kt=KT)
    wq_r = wq_d[:, :].rearrange("(kt p) c -> p kt c", p=P)
    wk_r = wk_d[:, :].rearrange("(kt p) c -> p kt c", p=P)
    wv_r = wv_d[:, :].rearrange("(kt p) c -> p kt c", p=P)

    with tile.TileContext(nc) as tc, ExitStack() as ctx:
        sing = ctx.enter_context(tc.tile_pool(name="sing", bufs=1))
        spool = ctx.enter_context(
            tc.tile_pool(name="s_ps", bufs=3, space=bass.MemorySpace.PSUM)
        )
        opool = ctx.enter_context(
            tc.tile_pool(name="o_ps", bufs=1, space=bass.MemorySpace.PSUM)
        )
        apool = ctx.enter_context(tc.tile_pool(name="a_sb", bufs=14))
        copool = ctx.enter_context(tc.tile_pool(name="o_sb", bufs=4))

        # persistent SBUF tensors
        xT = sing.tile([P, KT, N], bf16, tag="xT")
        wq = sing.tile([P, KT, HL * DH], bf16, tag="wq")
        wk = sing.tile([P, KT, HL * DH], bf16, tag="wk")
        wv = sing.tile([P, KT, HL * DH], bf16, tag="wv")
        # head-pair packed projections: partitions 0-63 head A dims, 64-127 head B
        qT = [sing.tile([P, N], bf16, tag=f"qT{i}", name=f"qT{i}") for i in range(2)]
        kT = [sing.tile([P, N], bf16, tag=f"kT{i}", name=f"kT{i}") for i in range(2)]
        # V in [j, d] layout per j-block per head, with ones column at d=64
        v = sing.tile([P, NT, HL, DH + 1], bf16, tag="v")

        # input DMAs
        nc.gpsimd.dma_start(out=wk[:], in_=wk_r[:])
        nc.gpsimd.dma_start(out=wq[:], in_=wq_r[:])
        nc.gpsimd.dma_start(out=wv[:], in_=wv_r[:])
        for c in range(4):
            nc.sync.dma_start(
                out=xT[:, 2 * c : 2 * c + 2, :], in_=xT_r[:, 2 * c : 2 * c + 2, :]
            )



        # ---- projections ----
        # k, q: out[c, n] = w[:, c].T @ xT.  hp0 upfront; hp1 woven into
        # attention-hp0's periods (PE fills slack while ACT runs exp).
        def proj_unit(wt, dst, hp, nch):
            """Emit the 8 K-accumulated matmuls + copy for one 512-col chunk,
            returned as two 4-matmul halves so weaving stays fine-grained."""
            state = {}

            def half(h):
                if h == 0:
                    state["ps"] = spool.tile([P, 512], f32, tag="sp", name="ps")
                ps = state["ps"]
                for kt in range(4 * h, 4 * h + 4):
                    nc.tensor.matmul(
                        ps[:],
                        wt[:, kt, hp * P : (hp + 1) * P],
                        xT[:, kt, nch * 512 : (nch + 1) * 512],
                        start=(kt == 0),
                        stop=(kt == KT - 1),
                    )
                if h == 1:
                    nc.vector.tensor_copy(dst[:, nch * 512 : (nch + 1) * 512], ps[:])

            return [lambda: half(0), lambda: half(1)]

        # PE warm-up: zero matmuls while the xT DMA streams in, so the HAM
        # clock-gate is at 8/8 when real work starts (idle PE decays to 1.2GHz)
        zbuf = sing.tile([P, 512], bf16, tag="zbuf")
        nc.vector.memset(zbuf[:], 0.0)
        # ones column of V (softmax denominator comes out of the PV matmul)
        nc.vector.memset(v[:, :, :, DH : DH + 1], 1.0)
        zp = spool.tile([P, 512], f32, tag="sp", name="zp")
        for _ in range(40):
            nc.tensor.matmul(zp[:], zbuf[:, 0:P], zbuf[:], start=True, stop=True)

        for unit in [proj_unit(wk, kT[0], 0, 0), proj_unit(wq, qT[0], 0, 0)]:
            for work in unit:
                work()

        # remaining projections are woven into the attention periods; each
        # woven chunk lands (in emission order) before the first scores
        # matmul that reads it.
        def full_unit(halves):
            return lambda: [h() for h in halves]

        woven = [full_unit(proj_unit(wq, qT[0], 0, 1))]
        woven_rest = []
        for nch in range(2, NCH):
            woven_rest.append(full_unit(proj_unit(wq, qT[0], 0, nch)))
        for wt, dst in ((wk, kT[1]), (wq, qT[1])):
            for nch in range(NCH):
                woven_rest.append(full_unit(proj_unit(wt, dst, 1, nch)))

        # V: out[n, c] = xT[:, ntile].T @ wv   -> [128 n, 256 c]
        def v_unit(nt):
            state = {}

            def half(h):
                if h == 0:
                    state["ps"] = spool.tile([P, HL * DH], f32, tag="sp", name="psv")
                ps = state["ps"]
                for kt in range(4 * h, 4 * h + 4):
                    nc.tensor.matmul(
                        ps[:],
                        xT[:, kt, nt * P : (nt + 1) * P],
                        wv[:, kt, :],
                        start=(kt == 0),
                        stop=(kt == KT - 1),
                    )
                if h == 1:
                    # scatter the 4 heads' 64 cols into the [NT, HL, 65] layout
                    nc.vector.tensor_copy(
                        v[:, nt, :, 0:DH],
                        ps[:].rearrange("p (h d) -> p h d", h=HL),
                    )

            return [lambda: half(0), lambda: half(1)]

        v_units = [full_unit(v_unit(nt)) for nt in range(NT)]

        # ---- attention ----
        # 8 blocks of 16 periods (one per (hp, ic)).  ACT runs one
        # [128, 1024] exp per period back-to-back; PE emits scores two
        # periods ahead (spool rotation) plus woven projection work; PV runs
        # as dense 8-matmul bursts every 4 periods (no exp-latency exposure).
        # Block 0 weaves the V projection (PV bursts shifted late until V is
        # ready); blocks 1+ weave the remaining q/k projections.
        blocks = [(hp, ic) for hp in range(2) for ic in range(NCH)]
        ats = {}
        opairs = {}
        sp_ahead = {}

        def emit_scores(b, jb):
            hp, ic = blocks[b]
            i0, j0 = ic * 512, jb * P
            sp = spool.tile([P, 1024], f32, tag="sp", name="sp")
            nc.tensor.matmul(
                sp[:, 0:512],
                kT[hp][0:DH, j0 : j0 + P],
                qT[hp][0:DH, i0 : i0 + 512],
                start=True, stop=True, tile_position=(0, 0),
            )
            nc.tensor.matmul(
                sp[:, 512:1024],
                kT[hp][DH:P, j0 : j0 + P],
                qT[hp][DH:P, i0 : i0 + 512],
                start=True, stop=True, tile_position=(64, 0),
            )
            return sp

        def emit_exp(b, jb, sp):
            at = apool.tile([P, 1024], bf16, tag="at", name="at")
            nc.scalar.activation(at[:], sp[:], mybir.ActivationFunctionType.Exp)
            ats[(b, jb)] = at

        def fetch_scores(b, jb):
            key = (b, jb)
            if key in sp_ahead:
                return sp_ahead.pop(key)
            return emit_scores(b, jb)

        def emit_pv_quarter(b, q):
            """PV matmuls for periods 4q..4q+3 of block b (dense burst)."""
            hp, ic = blocks[b]
            if q == 0:
                opairs[b] = (
                    opool.tile([DH + 1, 512], f32, tag="oA", name="oA"),
                    opool.tile([DH + 1, 512], f32, tag="oB", name="oB"),
                )
            oA, oB = opairs[b]
            for col, o in ((0, oA), (1, oB)):
                for jb in range(4 * q, 4 * q + 4):
                    nc.tensor.matmul(
                        o[:],
                        v[:, jb, 2 * hp + col, :],
                        ats[(b, jb)][:, 512 * col : 512 * col + 512],
                        start=(jb == 0), stop=(jb == NT - 1),
                    )
            for jb in range(4 * q, 4 * q + 4):
                del ats[(b, jb)]
            if q == 3:
                i0 = ic * 512
                os = copool.tile([DH + 1, 2, 512], f32, tag="os", name="os")
                nc.vector.tensor_copy(os[:, 0, :], oA[:])
                nc.vector.tensor_copy(os[:, 1, :], oB[:])
                nc.sync.dma_start(
                    out=out_r[:, 2 * hp : 2 * hp + 2, i0 : i0 + 512],
                    in_=os[:],
                )

        LA = 2  # scores lookahead depth
        nblocks = len(blocks)
        # prime the pipeline, then finish the kT01 projection chunks so the
        # first exp only waits on k01n0 + q01n0
        for j in range(LA):
            sp_ahead[(0, j)] = emit_scores(0, j)
        for nch in range(1, NCH):
            for work in proj_unit(wk, kT[0], 0, nch):
                work()
        for b in range(nblocks):
            for jb in range(NT):
                emit_exp(b, jb, fetch_scores(b, jb))
                la = jb + LA
                if la < NT:
                    sp_ahead[(b, la)] = emit_scores(b, la)
                elif b + 1 < nblocks:
                    sp_ahead[(b + 1, la - NT)] = emit_scores(b + 1, la - NT)
                # woven PE filler
                p = b * NT + jb
                if b == 0:
                    if woven:
                        woven.pop(0)()
                    for _ in range(2):
                        if not woven and v_units:
                            v_units.pop(0)()
                elif woven_rest and (p - NT) % 5 == 4:
                    woven_rest.pop(0)()
                # PV bursts (block 0 deferred until woven V is ready; each
                # block's last quarter runs in the next block's first period
                # so the boundary scores lookahead isn't delayed)
                if b == 0:
                    if jb in (7, 11):
                        emit_pv_quarter(0, (jb - 7) // 4)
                    elif jb == NT - 1:
                        while v_units:
                            v_units.pop(0)()
                        emit_pv_quarter(0, 2)
                        emit_pv_quarter(0, 3)
                elif jb % 4 == 3:
                    emit_pv_quarter(b, jb // 4)

    nc.compile()
    return nc


def _get_nc():
    if "nc" not in _CACHE:
        _CACHE["nc"] = _build_nc()
    return _CACHE["nc"]


def _prepare_in_maps(x, w_qkv):
    bf = ml_dtypes.bfloat16
    x = np.asarray(x, dtype=np.float32)
    w = np.asarray(w_qkv, dtype=np.float32)
    scale = DH ** -0.5
    in_maps = []
    xT_b = [
        np.ascontiguousarray(
            x[b].T.reshape(KT, P, N).transpose(1, 0, 2).reshape(P, KT * N)
        ).astype(bf)
        for b in range(B)
    ]
    for c in range(8):
        b, hg = divmod(c, 4)
        cs = slice(hg * HL * DH, (hg + 1) * HL * DH)
        in_maps.append(
            {
                "xT": xT_b[b],
                "wq": np.ascontiguousarray(w[:, cs] * scale).astype(bf),
                "wk": np.ascontiguousarray(w[:, 1024:2048][:, cs]).astype(bf),
                "wv": np.ascontiguousarray(w[:, 2048:3072][:, cs]).astype(bf),
            }
        )
    return in_maps


def _assemble(outs):
    full = np.empty((B, N, HEADS * DH), dtype=np.float32)
    for c in range(8):
        b, hg = divmod(c, 4)
        o = outs[c].reshape(HL, DH + 1, N)
        norm = o[:, :DH, :] / o[:, DH : DH + 1, :]  # [hl, d, n]
        full[b, :, hg * HL * DH : (hg + 1) * HL * DH] = norm.transpose(2, 0, 1).reshape(
            N, HL * DH
        )
    return full


def kernel(x, w_qkv):
    global LAST_RESULTS
    from concourse.bass_utils import run_bass_kernel_spmd

    nc = _get_nc()
    in_maps = _prepare_in_maps(x, w_qkv)
    res = run_bass_kernel_spmd(
        nc,
        in_maps,
        core_ids=list(range(8)),
        trace=TRACE,
        trace_cores=[0] if TRACE else None,
    )
    LAST_RESULTS = res
    return _assemble([r["out"] for r in res.results])
